# revision 13
# baseline (speedup 1.0000x reference)
"""CrossAttentionBlock kernel for 8 Trainium2 NeuronCores.

Reference computation (per batch b):
    q = x @ Wq;  k,v = y @ Wkv;  per head: softmax(q k^T / sqrt(dk)) v;
    out = concat_heads @ Wproj + bproj

Sharding: 8 cores = 2 batches x 4 head-groups (4 heads each). Each core
computes the partial output contribution of its 4 heads for its batch;
the host sums the 4 partials per batch and adds the bias.

All device I/O is bf16 (host converts inputs, sums bf16 output partials
in f32): halves DMA traffic vs f32 and lets every matmul run at the
1-cycle/row bf16 rate.

Per-core pipeline (16 attention tiles = 4 heads x 4 query-windows of 512):
  scores S^T [keys128, q512] (bf16 matmuls) -> exp into P^T slabs: slab 0
  (kv chunks 0-1) via a one-instruction Schraudolph on the DVE
  (i16 = round(s*SCALE*128/ln2 + 16248.5), bitcast bf16 == exp approx),
  the rest on the ACT engine (the near-critical resource) -> AV in
  query-major bf16: out[q128, 65] = P^T-slice.T @ [V | ones] with row-sums
  landing in column 64 for free; one PSUM accumulation group per bank
  (hardware `start` clears the whole bank) normalized immediately via DVE
  reciprocal + per-partition tensor_scalar -> bf16 PE transpose (via a
  DMA'd identity) back to feature-major -> bf16 output projection whose
  PSUM->SBUF copies alternate between ACT and DVE, partials summed on the
  host. Software-pipelined: each tile's epilogue is chunked and woven
  between the next tile's score slabs so ACT never starves; a PE warm-up
  filler keeps the sim's p-state warm through the DMA-bound lead-in.
"""

import numpy as np

import concourse.bass as bass
import concourse.tile as tile
from concourse import bacc, mybir
from concourse.bass_utils import run_bass_kernel_spmd

B, LQ, LKV = 2, 2048, 2048
C, CTX, H, DK = 1024, 768, 16, 64
SCALE = DK ** (-0.5)
HD = 256                 # head-group width (4 heads x 64)

F32 = mybir.dt.float32
BF16 = mybir.dt.bfloat16
I16 = mybir.dt.int16

# Schraudolph exp: bitcast(round(s*SA + SB)) as bf16 ~= exp(s*SCALE)
SA = SCALE * 128.0 / float(np.log(2.0))
SB = 127.0 * 128.0 - 7.5

NCC = C // 128           # x contraction chunks (8)
NCTX = CTX // 128        # y contraction chunks (6)
NIT = LQ // 512          # query windows (4)
NYQ = LKV // 256         # kv quarter-windows (8)
NJT = LKV // 128         # kv chunks (16)
# exp slab schedule over the 16 kv chunks: (start, len)
import os as _os
if _os.environ.get("K_SLAB8"):
    SLABS = [(2 * i, 2) for i in range(8)]
    STP_BUFS = 3
    STP_CH = 2
else:
    SLABS = [(0, 2), (2, 3), (5, 3), (8, 3), (11, 3), (14, 2)]
    STP_BUFS = 2
    STP_CH = 3
NSLAB = len(SLABS)
DVE_SLABS = tuple(int(c) for c in _os.environ.get("K_DVE_SLABS", ""))
OSB_ACT = tuple(int(c) for c in _os.environ.get("K_OSB_ACT", ""))
JT2SLAB = {}
for _si, (_g0, _g) in enumerate(SLABS):
    for _jt in range(_g0, _g0 + _g):
        JT2SLAB[_jt] = (_si, _jt - _g0)


def build_kernel(debug_taps=False):
    nc = bacc.Bacc("TRN2", target_bir_lowering=False, debug=False)

    xT = nc.dram_tensor("xT", [C, LQ], BF16, kind="ExternalInput").ap()
    yT = nc.dram_tensor("yT", [CTX, LKV], BF16, kind="ExternalInput").ap()
    wq = nc.dram_tensor("wq", [C, HD], BF16, kind="ExternalInput").ap()
    wk = nc.dram_tensor("wk", [CTX, HD], BF16, kind="ExternalInput").ap()
    wv = nc.dram_tensor("wv", [CTX, HD], BF16, kind="ExternalInput").ap()
    wp = nc.dram_tensor("wp", [HD, C], BF16, kind="ExternalInput").ap()
    idn_d = nc.dram_tensor("ident", [128, 128], BF16, kind="ExternalInput").ap()
    outT = nc.dram_tensor("outT", [C, LQ], BF16, kind="ExternalOutput").ap()
    taps = {}
    if debug_taps:
        taps["dbg_qt"] = nc.dram_tensor(
            "dbg_qt", [128, 2, LQ], BF16, kind="ExternalOutput").ap()
        taps["dbg_kt"] = nc.dram_tensor(
            "dbg_kt", [128, 2, LKV], BF16, kind="ExternalOutput").ap()
        taps["dbg_vaug"] = nc.dram_tensor(
            "dbg_vaug", [128, NJT, 4, 65], BF16, kind="ExternalOutput").ap()
        taps["dbg_otn"] = nc.dram_tensor(
            "dbg_otn", [128, 2, LQ], BF16, kind="ExternalOutput").ap()

    with tile.TileContext(nc) as tc:
        with (
            tc.tile_pool(name="wts", bufs=1) as wts,
            tc.tile_pool(name="acts", bufs=1) as acts,
            tc.tile_pool(name="xp", bufs=2) as xp,
            tc.tile_pool(name="yp", bufs=8) as yp,
            tc.tile_pool(name="ptp", bufs=NSLAB + 9) as ptp,
            tc.tile_pool(name="ostp", bufs=2) as ostp,
            tc.tile_pool(name="rsp", bufs=4) as rsp,
            tc.tile_pool(name="osbp", bufs=4) as osbp,
            tc.tile_pool(name="stp", bufs=STP_BUFS, space="PSUM") as stp,
            tc.tile_pool(name="mscp", bufs=2, space="PSUM") as mscp,
        ):
            # ---- persistent weights / activations
            wq_sb = wts.tile([128, NCC, HD], BF16, tag="wq")
            wk_sb = wts.tile([128, NCTX, HD], BF16, tag="wk")
            wv_sb = wts.tile([128, NCTX, HD], BF16, tag="wv")
            wp_sb = wts.tile([128, 2, C], BF16, tag="wp")
            idn = wts.tile([128, 128], BF16, tag="idn")
            ones_sb = wts.tile([128, NJT, 4], BF16, tag="ones")
            dummy = wts.tile([128, 128], BF16, tag="dummy")

            qt = acts.tile([128, 2, LQ], BF16, tag="qt")       # Q^T pair-stacked
            kt = acts.tile([128, 2, LKV], BF16, tag="kt")      # K^T pair-stacked
            vaug = acts.tile([128, NJT, 4, 65], BF16, tag="vaug")  # [V_h | 1]
            otn = acts.tile([128, 2, LQ], BF16, tag="otn")     # normalized O^T

            # pin the Exp act-table + bias const load to t~0 (their DMAs
            # would otherwise queue behind all the input loads)
            nc.gpsimd.memset(dummy[:], 0.0)
            nc.scalar.activation(
                dummy[:, 0:1], dummy[:, 1:2],
                mybir.ActivationFunctionType.Exp, scale=SCALE)
            nc.gpsimd.memset(ones_sb[:], 1.0)
            nc.gpsimd.tensor_copy(
                vaug[:, :, :, 64:65],
                ones_sb[:].rearrange("p j (h o) -> p j h o", o=1))

            # ---- input DMAs in priority order (SP queue drains in order);
            # tiles 0/1 are pair-0 heads, so pair-0 weight halves come first
            wq_r = wq.rearrange("(cc p) h -> p cc h", p=128)
            wk_r = wk.rearrange("(cc p) h -> p cc h", p=128)
            nc.sync.dma_start(out=wq_sb, in_=wq_r)

            def load_x(w, half=None):
                t = xp.tile([128, NCC, 512], BF16, tag="x", name=f"x{w}")
                src = xT.rearrange("(cc p) l -> p cc l", p=128)
                if half is None:
                    nc.sync.dma_start(
                        out=t, in_=src[:, :, w * 512:(w + 1) * 512])
                else:
                    nc.sync.dma_start(
                        out=t[:, :, 0:256],
                        in_=src[:, :, w * 512:w * 512 + 256])
                return t

            def load_x_half2(t, w):
                src = xT.rearrange("(cc p) l -> p cc l", p=128)
                nc.sync.dma_start(
                    out=t[:, :, 256:512],
                    in_=src[:, :, w * 512 + 256:(w + 1) * 512])

            def load_yq(q):
                t = yp.tile([128, NCTX, 256], BF16, tag="y", name=f"y{q}")
                nc.sync.dma_start(
                    out=t,
                    in_=yT.rearrange("(cc p) l -> p cc l", p=128)
                    [:, :, q * 256:(q + 1) * 256])
                return t

            x_t = [None] * NIT
            y_q = [None] * NYQ
            x_t[0] = load_x(0, half=0)
            nc.sync.dma_start(out=wk_sb, in_=wk_r)
            y_q[0] = load_yq(0)
            load_x_half2(x_t[0], 0)
            y_q[1] = load_yq(1)
            nc.sync.dma_start(
                out=wv_sb, in_=wv.rearrange("(cc p) h -> p cc h", p=128))
            for q in range(2, NYQ):
                y_q[q] = load_yq(q)
            nc.sync.dma_start(out=idn, in_=idn_d)
            x_t[1] = load_x(1)
            nc.sync.dma_start(
                out=wp_sb, in_=wp.rearrange("(r p) o -> p r o", p=128))
            x_t[2] = load_x(2)
            x_t[3] = load_x(3)

            # ---- PE warm-up filler: keeps the PE busy stretch alive through
            # the DMA-bound lead-in so real bursts are costed at full p-state
            dps = stp.tile([128, STP_CH, 512], F32, tag="st", name="dps")

            def filler(n, gate=None):
                src = dummy[0:64, 0:64] if gate is None else gate
                for _ in range(n):
                    nc.tensor.matmul(
                        dps[0:64, 0, 0:64], src, src,
                        start=True, stop=True, skip_group_check=True)

            filler(60)

            # ---- projection helpers (kv projections run per quarter-window)
            def kproj_p(q, pair):
                ps = mscp.tile([128, 256], F32, tag="ms", name=f"psk{q}{pair}")
                for cc in range(NCTX):
                    nc.tensor.matmul(
                        ps[:],
                        wk_sb[:, cc, pair * 128:(pair + 1) * 128],
                        y_q[q][:, cc, :],
                        start=(cc == 0), stop=(cc == NCTX - 1))
                nc.vector.tensor_copy(kt[:, pair, q * 256:(q + 1) * 256], ps[:])

            def vproj(q):
                for j in range(2):
                    jt = 2 * q + j
                    ps = mscp.tile([128, 256], F32, tag="ms", name=f"psv{jt}")
                    for cc in range(NCTX):
                        nc.tensor.matmul(
                            ps[:],
                            y_q[q][:, cc, j * 128:(j + 1) * 128],
                            wv_sb[:, cc, :],
                            start=(cc == 0), stop=(cc == NCTX - 1))
                    nc.vector.tensor_copy(
                        vaug[:, jt, :, 0:64],
                        ps[:].rearrange("p (h d) -> p h d", d=64))

            def qproj_half(it, pair, half):
                ps = mscp.tile([128, 256], F32, tag="ms", name=f"psqh{pair}{half}")
                for cc in range(NCC):
                    nc.tensor.matmul(
                        ps[:],
                        wq_sb[:, cc, pair * 128:(pair + 1) * 128],
                        x_t[it][:, cc, half * 256:half * 256 + 256],
                        start=(cc == 0), stop=(cc == NCC - 1))
                nc.vector.tensor_copy(
                    qt[:, pair, it * 512 + half * 256:it * 512 + half * 256 + 256],
                    ps[:])

            def qproj_pair(it, pair):
                ps = mscp.tile([128, 512], F32, tag="ms", name=f"psq{it}{pair}")
                for cc in range(NCC):
                    nc.tensor.matmul(
                        ps[:],
                        wq_sb[:, cc, pair * 128:(pair + 1) * 128],
                        x_t[it][:, cc, :],
                        start=(cc == 0), stop=(cc == NCC - 1))
                nc.vector.tensor_copy(qt[:, pair, it * 512:(it + 1) * 512], ps[:])

            # ---- attention stages
            tiles = [(h, it) for it in range(NIT) for h in range(4)]
            pt_slabs = {}     # (idx, si) -> P^T slab tile (bf16, or i16 slab 0)
            ot_tiles = {}
            ost_tiles = {}

            def sc_slab(idx, si):
                """Scores + exp for slab si of tile idx. Slab 0's exp runs on
                the DVE via Schraudolph; the rest on ACT. Runs at boosted
                scheduler priority: these feed the near-saturated engines."""
                h, it = tiles[idx]
                pair, hp = h // 2, h % 2
                base = hp * 64
                g0, glen = SLABS[si]
                with tc.high_priority(offset=8000):
                    st = stp.tile([128, STP_CH, 512], F32, tag="st",
                                  name=f"st{idx}_{si}")
                    for k in range(glen):
                        jt = g0 + k
                        nc.tensor.matmul(
                            st[:, k, :],
                            kt[base:base + 64, pair, jt * 128:(jt + 1) * 128],
                            qt[base:base + 64, pair, it * 512:(it + 1) * 512],
                            start=True, stop=True)
                    if si in DVE_SLABS:
                        pt = ptp.tile([128, STP_CH, 512], I16, tag="pt",
                                      name=f"pt{idx}_{si}")
                        nc.vector.tensor_scalar(
                            pt[:, 0:glen, :], st[:, 0:glen, :], SA, SB,
                            mybir.AluOpType.mult, mybir.AluOpType.add)
                        pt_slabs[(idx, si)] = pt[:].bitcast(BF16)
                    else:
                        pt = ptp.tile([128, STP_CH, 512], BF16, tag="pt",
                                      name=f"pt{idx}_{si}")
                        nc.scalar.activation(
                            pt[:, 0:glen, :], st[:, 0:glen, :],
                            mybir.ActivationFunctionType.Exp, scale=SCALE)
                        pt_slabs[(idx, si)] = pt

            def chunk_av_qb(idx, qb):
                """AV accumulation for one 128-query block: a single PSUM
                accumulation group per bank (hardware `start` clears the whole
                bank, so groups must not interleave within one), normalized
                immediately so the pool slot recycles."""
                h, it = tiles[idx]
                pair, hp = h // 2, h % 2
                if hp == 0 and qb == 0:
                    ost_tiles[(pair, it)] = ostp.tile(
                        [128, 4, 128], BF16, tag="ost", name=f"ost{pair}{it}")
                ost = ost_tiles[(pair, it)]
                ot = mscp.tile([128, 65], F32, tag="ms", name=f"ot{idx}_{qb}")
                for jt in range(NJT):
                    si, k = JT2SLAB[jt]
                    nc.tensor.matmul(
                        ot[:],
                        pt_slabs[(idx, si)][:, k, qb * 128:(qb + 1) * 128],
                        vaug[:, jt, h, :],
                        start=(jt == 0), stop=(jt == NJT - 1))
                rs = rsp.tile([128, 1], F32, tag="rs", name=f"rs{idx}{qb}")
                nc.vector.reciprocal(out=rs[:], in_=ot[:, 64:65])
                nc.vector.tensor_scalar_mul(
                    ost[:, qb, hp * 64:(hp + 1) * 64], ot[:, 0:64], rs[:])

            def chunk_transposes(idx, qbs=range(4)):
                h, it = tiles[idx]
                if h % 2 != 1:
                    return
                pair = h // 2
                ost = ost_tiles[(pair, it)]
                for qb in qbs:
                    tp = mscp.tile([128, 128], BF16, tag="ms", name=f"tp{idx}{qb}")
                    nc.tensor.transpose(tp[:], ost[:, qb, :], idn[:])
                    nc.vector.tensor_copy(
                        otn[:, pair, it * 512 + qb * 128:it * 512 + (qb + 1) * 128],
                        tp[:])

            def chunk_qproj(idx):
                h, it = tiles[idx]
                if h % 2 == 1 and it + 1 < NIT:
                    qproj_pair(it + 1, h // 2)

            def outproj_quarter(it, cts):
                for ct in cts:
                    ps = mscp.tile([128, 512], F32, tag="ms", name=f"psp{it}{ct}")
                    for r in range(2):
                        nc.tensor.matmul(
                            ps[:],
                            wp_sb[:, r, ct * 128:(ct + 1) * 128],
                            otn[:, r, it * 512:(it + 1) * 512],
                            start=(r == 0), stop=(r == 1))
                    o_sb = osbp.tile([128, 512], BF16, tag="osb", name=f"osb{it}{ct}")
                    # PSUM->SBUF copy: ACT for selected cts, else DVE
                    if ct % 2 in OSB_ACT:
                        nc.scalar.activation(
                            o_sb[:], ps[:], mybir.ActivationFunctionType.Copy)
                    else:
                        nc.vector.tensor_copy(o_sb[:], ps[:])
                    nc.sync.dma_start(
                        out=outT[ct * 128:(ct + 1) * 128, it * 512:(it + 1) * 512],
                        in_=o_sb[:])

            # output projections are deferred into the following (lighter)
            # tiles' chunk slots so the ACT-feeding score matmuls of heavy
            # tiles aren't crowded out
            defer = []

            def chunk_deferred():
                if defer:
                    defer.pop(0)()

            def chunk_tp_qp(idx):
                h, it = tiles[idx]
                chunk_transposes(idx)
                chunk_qproj(idx)
                if h == 3:
                    # safe to enqueue only now: outproj(it) must be emitted
                    # after this tile's transposes (PSUM pool WAR cycle)
                    for cts in ([0, 1], [2, 3], [4, 5], [6, 7]):
                        defer.append(lambda it=it, cts=cts: outproj_quarter(it, cts))

            def epilogue_chunks(idx):
                chunks = [
                    lambda: chunk_av_qb(idx, 0),
                    lambda: chunk_av_qb(idx, 1),
                    lambda: chunk_av_qb(idx, 2),
                    lambda: (chunk_av_qb(idx, 3), chunk_deferred()),
                    lambda: chunk_tp_qp(idx),
                    lambda: chunk_deferred(),
                ]
                while len(chunks) < NSLAB:
                    chunks.append(lambda: chunk_deferred())
                return chunks

            # ---- lead-in: Q proj of window 0, K per kv quarter as it
            # arrives, first two tiles' score slabs right behind (the ACT
            # engine is the critical resource — feed it ASAP); V projections
            # are deferred/spread since vaug is first read only at AV(0)
            qproj_half(0, 0, 0)
            filler(50)
            kproj_p(0, 0)
            filler(40)
            qproj_half(0, 0, 1)
            filler(20)
            kproj_p(1, 0)
            # kt quarters needed per slab si: last jt of the slab / 2
            slab_qhi = [(g0 + g - 1) * 128 // 256 for (g0, g) in SLABS]
            vq_done = 0
            kq_done = 2
            for si in range(NSLAB - 1):
                # the K-proj -> scores chain feeds ACT (the critical engine);
                # boost its scheduler priority over the V-projection backfill
                with tc.high_priority(offset=3000):
                    while kq_done <= slab_qhi[si]:
                        kproj_p(kq_done, 0)
                        kq_done += 1
                    sc_slab(0, si)
                    sc_slab(1, si)
                while vq_done < min(kq_done, 2 * si + 2, NYQ):
                    vproj(vq_done)
                    vq_done += 1
            while kq_done <= slab_qhi[NSLAB - 1]:
                kproj_p(kq_done, 0)
                kq_done += 1
            while vq_done < NYQ:
                vproj(vq_done)
                vq_done += 1
            sc_slab(0, NSLAB - 1)        # last kv chunks
            chunk_av_qb(0, 0)
            chunk_av_qb(0, 1)
            chunk_av_qb(0, 2)
            chunk_av_qb(0, 3)
            sc_slab(1, NSLAB - 1)
            # pair-1 projections (needed from tile 2 on): Q at lead end,
            # K woven just ahead of the tile-2 slabs that consume them
            qproj_half(0, 1, 0)
            qproj_half(0, 1, 1)
            if NSLAB == 6:
                kp1_sched = {(2, 0): [0, 1], (2, 1): [2], (2, 2): [3, 4],
                             (2, 3): [5], (2, 4): [6, 7]}
            else:
                kp1_sched = {(2, 0): [0, 1]}
                for _si in range(1, 7):
                    kp1_sched[(2, _si)] = [_si + 1]

            # ---- steady-state: weave tile idx-1's epilogue chunks between
            # tile idx's score slabs
            for idx in range(2, len(tiles) + 1):
                chunks = epilogue_chunks(idx - 1)
                for si in range(NSLAB):
                    for q in kp1_sched.get((idx, si), []):
                        kproj_p(q, 1)
                    if idx < len(tiles):
                        sc_slab(idx, si)
                    chunks[si]()
            while defer:
                defer.pop(0)()
            if debug_taps:
                nc.sync.dma_start(out=taps["dbg_qt"], in_=qt[:])
                nc.sync.dma_start(out=taps["dbg_kt"], in_=kt[:])
                nc.sync.dma_start(out=taps["dbg_vaug"], in_=vaug[:])
                nc.sync.dma_start(out=taps["dbg_otn"], in_=otn[:])

    nc.compile()
    return nc


_NC_CACHE = {}


def _get_nc():
    if "nc" not in _NC_CACHE:
        _NC_CACHE["nc"] = build_kernel()
    return _NC_CACHE["nc"]


def _bf16(a):
    """Round-to-nearest-even f32 -> bf16 (stored as uint16 bit pattern
    viewed via numpy's void trick is messy; ml_dtypes handles it)."""
    import ml_dtypes
    return np.asarray(a, dtype=np.float32).astype(ml_dtypes.bfloat16)


def make_in_maps(x, y, Wq, Wkv, Wproj):
    """Host-side sharding: core = b * 4 + hg (hg = 4-head group)."""
    x = np.asarray(x, dtype=np.float32)
    y = np.asarray(y, dtype=np.float32)
    Wq = np.asarray(Wq, dtype=np.float32)
    Wkv = np.asarray(Wkv, dtype=np.float32).reshape(CTX, 2, H, DK)
    Wproj = np.asarray(Wproj, dtype=np.float32)
    ident = np.eye(128, dtype=np.float32)

    in_maps = []
    for core in range(8):
        b, hg = core // 4, core % 4
        hs = slice(4 * hg, 4 * hg + 4)
        in_maps.append({
            "xT": _bf16(np.ascontiguousarray(x[b].T)),
            "yT": _bf16(np.ascontiguousarray(y[b].T)),
            "wq": _bf16(np.ascontiguousarray(Wq[:, 4 * hg * DK:(4 * hg + 4) * DK])),
            "wk": _bf16(np.ascontiguousarray(Wkv[:, 0, hs, :].reshape(CTX, 4 * DK))),
            "wv": _bf16(np.ascontiguousarray(Wkv[:, 1, hs, :].reshape(CTX, 4 * DK))),
            "wp": _bf16(np.ascontiguousarray(Wproj[4 * hg * DK:(4 * hg + 4) * DK, :])),
            "ident": _bf16(ident),
        })
    return in_maps


def kernel(x, y, Wq, Wkv, Wproj, bproj):
    nc = _get_nc()
    in_maps = make_in_maps(x, y, Wq, Wkv, Wproj)
    res = run_bass_kernel_spmd(nc, in_maps, core_ids=list(range(8)))
    bproj = np.asarray(bproj, dtype=np.float32)
    out = np.empty((B, LQ, C), dtype=np.float32)
    for b in range(B):
        acc = res.results[4 * b]["outT"].astype(np.float32)
        for hg in range(1, 4):
            acc = acc + res.results[4 * b + hg]["outT"].astype(np.float32)
        out[b] = acc.T + bproj
    return out


# revision 15
# speedup vs baseline: 1.0799x; 1.0799x over previous
"""CrossAttentionBlock kernel for 8 Trainium2 NeuronCores.

Reference computation (per batch b):
    q = x @ Wq;  k,v = y @ Wkv;  per head: softmax(q k^T / sqrt(dk)) v;
    out = concat_heads @ Wproj + bproj

Sharding: 8 cores = 2 batches x 4 head-groups (4 heads each). Each core
computes the partial output contribution of its 4 heads for its batch;
the host sums the 4 partials per batch and adds the bias.

All device I/O is bf16 (host converts inputs, sums bf16 output partials
in f32): halves DMA traffic vs f32 and lets every matmul run at the
1-cycle/row bf16 rate.

Per-core pipeline (16 attention tiles = 4 heads x 4 query-windows of 512):
  scores S^T [keys128, q512] (bf16 matmuls) -> exp into P^T slabs: slab 0
  (kv chunks 0-1) via a one-instruction Schraudolph on the DVE
  (i16 = round(s*SCALE*128/ln2 + 16248.5), bitcast bf16 == exp approx),
  the rest on the ACT engine (the near-critical resource) -> AV in
  query-major bf16: out[q128, 65] = P^T-slice.T @ [V | ones] with row-sums
  landing in column 64 for free; one PSUM accumulation group per bank
  (hardware `start` clears the whole bank) normalized immediately via DVE
  reciprocal + per-partition tensor_scalar -> bf16 PE transpose (via a
  DMA'd identity) back to feature-major -> bf16 output projection whose
  PSUM->SBUF copies alternate between ACT and DVE, partials summed on the
  host. Software-pipelined: each tile's epilogue is chunked and woven
  between the next tile's score slabs so ACT never starves; a PE warm-up
  filler keeps the sim's p-state warm through the DMA-bound lead-in.
"""

import numpy as np

import concourse.bass as bass
import concourse.tile as tile
from concourse import bacc, mybir
from concourse.bass_utils import run_bass_kernel_spmd

B, LQ, LKV = 2, 2048, 2048
C, CTX, H, DK = 1024, 768, 16, 64
SCALE = DK ** (-0.5)
HD = 256                 # head-group width (4 heads x 64)

F32 = mybir.dt.float32
BF16 = mybir.dt.bfloat16
I16 = mybir.dt.int16

# Schraudolph exp: bitcast(round(s*SA + SB)) as bf16 ~= exp(s*SCALE)
SA = SCALE * 128.0 / float(np.log(2.0))
SB = 127.0 * 128.0 - 7.5

NCC = C // 128           # x contraction chunks (8)
NCTX = CTX // 128        # y contraction chunks (6)
NIT = LQ // 512          # query windows (4)
NYQ = LKV // 256         # kv quarter-windows (8)
NJT = LKV // 128         # kv chunks (16)
# exp slab schedule over the 16 kv chunks: (start, len)
import os as _os
if _os.environ.get("K_SLAB8"):
    SLABS = [(2 * i, 2) for i in range(8)]
    STP_BUFS = 3
    STP_CH = 2
else:
    SLABS = [(0, 2), (2, 3), (5, 3), (8, 3), (11, 3), (14, 2)]
    STP_BUFS = 2
    STP_CH = 3
NSLAB = len(SLABS)
DVE_SLABS = tuple(int(c) for c in _os.environ.get("K_DVE_SLABS", ""))
OSB_ACT = tuple(int(c) for c in _os.environ.get("K_OSB_ACT", ""))
JT2SLAB = {}
for _si, (_g0, _g) in enumerate(SLABS):
    for _jt in range(_g0, _g0 + _g):
        JT2SLAB[_jt] = (_si, _jt - _g0)


def build_kernel(debug_taps=False):
    nc = bacc.Bacc("TRN2", target_bir_lowering=False, debug=False)

    xT = nc.dram_tensor("xT", [C, LQ], BF16, kind="ExternalInput").ap()
    yT = nc.dram_tensor("yT", [CTX, LKV], BF16, kind="ExternalInput").ap()
    wq = nc.dram_tensor("wq", [C, HD], BF16, kind="ExternalInput").ap()
    wk = nc.dram_tensor("wk", [CTX, HD], BF16, kind="ExternalInput").ap()
    wv = nc.dram_tensor("wv", [CTX, HD], BF16, kind="ExternalInput").ap()
    wp = nc.dram_tensor("wp", [HD, C], BF16, kind="ExternalInput").ap()
    idn_d = nc.dram_tensor("ident", [128, 128], BF16, kind="ExternalInput").ap()
    outT = nc.dram_tensor("outT", [C, LQ], BF16, kind="ExternalOutput").ap()
    taps = {}
    if debug_taps:
        taps["dbg_qt"] = nc.dram_tensor(
            "dbg_qt", [128, 2, LQ], BF16, kind="ExternalOutput").ap()
        taps["dbg_kt"] = nc.dram_tensor(
            "dbg_kt", [128, 2, LKV], BF16, kind="ExternalOutput").ap()
        taps["dbg_vaug"] = nc.dram_tensor(
            "dbg_vaug", [128, NJT, 4, 65], BF16, kind="ExternalOutput").ap()
        taps["dbg_otn"] = nc.dram_tensor(
            "dbg_otn", [128, 2, LQ], BF16, kind="ExternalOutput").ap()

    with tile.TileContext(nc) as tc:
        with (
            tc.tile_pool(name="wts", bufs=1) as wts,
            tc.tile_pool(name="acts", bufs=1) as acts,
            tc.tile_pool(name="xp", bufs=2) as xp,
            tc.tile_pool(name="yp", bufs=8) as yp,
            tc.tile_pool(name="ptp", bufs=NSLAB + 9) as ptp,
            tc.tile_pool(name="ostp", bufs=2) as ostp,
            tc.tile_pool(name="rsp", bufs=4) as rsp,
            tc.tile_pool(name="osbp", bufs=4) as osbp,
            tc.tile_pool(name="stp", bufs=STP_BUFS, space="PSUM") as stp,
            tc.tile_pool(name="mscp", bufs=2, space="PSUM") as mscp,
        ):
            # ---- persistent weights / activations
            wq_sb = wts.tile([128, NCC, HD], BF16, tag="wq")
            wk_sb = wts.tile([128, NCTX, HD], BF16, tag="wk")
            wv_sb = wts.tile([128, NCTX, HD], BF16, tag="wv")
            wp_sb = wts.tile([128, 2, C], BF16, tag="wp")
            idn = wts.tile([128, 128], BF16, tag="idn")
            ones_sb = wts.tile([128, NJT, 4], BF16, tag="ones")
            dummy = wts.tile([128, 128], BF16, tag="dummy")

            qt = acts.tile([128, 2, LQ], BF16, tag="qt")       # Q^T pair-stacked
            kt = acts.tile([128, 2, LKV], BF16, tag="kt")      # K^T pair-stacked
            vaug = acts.tile([128, NJT, 4, 65], BF16, tag="vaug")  # [V_h | 1]
            otn = acts.tile([128, 2, LQ], BF16, tag="otn")     # normalized O^T

            # pin the Exp act-table + bias const load to t~0 (their DMAs
            # would otherwise queue behind all the input loads)
            nc.gpsimd.memset(dummy[:], 0.0)
            nc.scalar.activation(
                dummy[:, 0:1], dummy[:, 1:2],
                mybir.ActivationFunctionType.Exp, scale=SCALE)
            nc.gpsimd.memset(ones_sb[:], 1.0)
            nc.gpsimd.tensor_copy(
                vaug[:, :, :, 64:65],
                ones_sb[:].rearrange("p j (h o) -> p j h o", o=1))

            # ---- input DMAs in priority order (SP queue drains in order);
            # tiles 0/1 are pair-0 heads, so pair-0 weight halves come first
            wq_r = wq.rearrange("(cc p) h -> p cc h", p=128)
            wk_r = wk.rearrange("(cc p) h -> p cc h", p=128)
            nc.sync.dma_start(out=wq_sb, in_=wq_r)

            def load_x(w, half=None):
                t = xp.tile([128, NCC, 512], BF16, tag="x", name=f"x{w}")
                src = xT.rearrange("(cc p) l -> p cc l", p=128)
                if half is None:
                    nc.sync.dma_start(
                        out=t, in_=src[:, :, w * 512:(w + 1) * 512])
                else:
                    nc.sync.dma_start(
                        out=t[:, :, 0:256],
                        in_=src[:, :, w * 512:w * 512 + 256])
                return t

            def load_x_half2(t, w):
                src = xT.rearrange("(cc p) l -> p cc l", p=128)
                nc.sync.dma_start(
                    out=t[:, :, 256:512],
                    in_=src[:, :, w * 512 + 256:(w + 1) * 512])

            def load_yq(q):
                t = yp.tile([128, NCTX, 256], BF16, tag="y", name=f"y{q}")
                nc.sync.dma_start(
                    out=t,
                    in_=yT.rearrange("(cc p) l -> p cc l", p=128)
                    [:, :, q * 256:(q + 1) * 256])
                return t

            x_t = [None] * NIT
            y_q = [None] * NYQ
            x_t[0] = load_x(0, half=0)
            nc.sync.dma_start(out=wk_sb, in_=wk_r)
            y_q[0] = load_yq(0)
            load_x_half2(x_t[0], 0)
            y_q[1] = load_yq(1)
            nc.sync.dma_start(
                out=wv_sb, in_=wv.rearrange("(cc p) h -> p cc h", p=128))
            for q in range(2, NYQ):
                y_q[q] = load_yq(q)
            nc.sync.dma_start(out=idn, in_=idn_d)
            x_t[1] = load_x(1)
            nc.sync.dma_start(
                out=wp_sb, in_=wp.rearrange("(r p) o -> p r o", p=128))
            x_t[2] = load_x(2)
            x_t[3] = load_x(3)

            # ---- PE warm-up filler: keeps the PE busy stretch alive through
            # the DMA-bound lead-in so real bursts are costed at full p-state
            dps = stp.tile([128, STP_CH, 512], F32, tag="st", name="dps")

            def filler(n, gate=None):
                src = dummy[0:64, 0:64] if gate is None else gate
                for _ in range(n):
                    nc.tensor.matmul(
                        dps[0:64, 0, 0:64], src, src,
                        start=True, stop=True, skip_group_check=True)

            filler(60)

            # ---- projection helpers (kv projections run per quarter-window)
            def kproj_p(q, pair):
                ps = mscp.tile([128, 256], F32, tag="ms", name=f"psk{q}{pair}")
                for cc in range(NCTX):
                    nc.tensor.matmul(
                        ps[:],
                        wk_sb[:, cc, pair * 128:(pair + 1) * 128],
                        y_q[q][:, cc, :],
                        start=(cc == 0), stop=(cc == NCTX - 1))
                nc.vector.tensor_copy(kt[:, pair, q * 256:(q + 1) * 256], ps[:])

            def vproj(q):
                for j in range(2):
                    jt = 2 * q + j
                    ps = mscp.tile([128, 256], F32, tag="ms", name=f"psv{jt}")
                    for cc in range(NCTX):
                        nc.tensor.matmul(
                            ps[:],
                            y_q[q][:, cc, j * 128:(j + 1) * 128],
                            wv_sb[:, cc, :],
                            start=(cc == 0), stop=(cc == NCTX - 1))
                    nc.vector.tensor_copy(
                        vaug[:, jt, :, 0:64],
                        ps[:].rearrange("p (h d) -> p h d", d=64))

            def qproj_half(it, pair, half):
                ps = mscp.tile([128, 256], F32, tag="ms", name=f"psqh{pair}{half}")
                for cc in range(NCC):
                    nc.tensor.matmul(
                        ps[:],
                        wq_sb[:, cc, pair * 128:(pair + 1) * 128],
                        x_t[it][:, cc, half * 256:half * 256 + 256],
                        start=(cc == 0), stop=(cc == NCC - 1))
                nc.vector.tensor_copy(
                    qt[:, pair, it * 512 + half * 256:it * 512 + half * 256 + 256],
                    ps[:])

            def qproj_pair(it, pair):
                ps = mscp.tile([128, 512], F32, tag="ms", name=f"psq{it}{pair}")
                for cc in range(NCC):
                    nc.tensor.matmul(
                        ps[:],
                        wq_sb[:, cc, pair * 128:(pair + 1) * 128],
                        x_t[it][:, cc, :],
                        start=(cc == 0), stop=(cc == NCC - 1))
                nc.vector.tensor_copy(qt[:, pair, it * 512:(it + 1) * 512], ps[:])

            # ---- attention stages
            tiles = [(h, it) for it in range(NIT) for h in range(4)]
            pt_slabs = {}     # (idx, si) -> P^T slab tile (bf16, or i16 slab 0)
            ot_tiles = {}
            ost_tiles = {}

            def sc_slab(idx, si):
                """Scores + exp for slab si of tile idx. Slab 0's exp runs on
                the DVE via Schraudolph; the rest on ACT. Runs at boosted
                scheduler priority: these feed the near-saturated engines."""
                h, it = tiles[idx]
                pair, hp = h // 2, h % 2
                base = hp * 64
                g0, glen = SLABS[si]
                with tc.high_priority(offset=8000):
                    st = stp.tile([128, STP_CH, 512], F32, tag="st",
                                  name=f"st{idx}_{si}")
                    for k in range(glen):
                        jt = g0 + k
                        nc.tensor.matmul(
                            st[:, k, :],
                            kt[base:base + 64, pair, jt * 128:(jt + 1) * 128],
                            qt[base:base + 64, pair, it * 512:(it + 1) * 512],
                            start=True, stop=True)
                    if si in DVE_SLABS:
                        pt = ptp.tile([128, STP_CH, 512], I16, tag="pt",
                                      name=f"pt{idx}_{si}")
                        nc.vector.tensor_scalar(
                            pt[:, 0:glen, :], st[:, 0:glen, :], SA, SB,
                            mybir.AluOpType.mult, mybir.AluOpType.add)
                        pt_slabs[(idx, si)] = pt[:].bitcast(BF16)
                    else:
                        pt = ptp.tile([128, STP_CH, 512], BF16, tag="pt",
                                      name=f"pt{idx}_{si}")
                        nc.scalar.activation(
                            pt[:, 0:glen, :], st[:, 0:glen, :],
                            mybir.ActivationFunctionType.Exp, scale=SCALE)
                        pt_slabs[(idx, si)] = pt

            def chunk_av_qb(idx, qb):
                """AV accumulation for one 128-query block: a single PSUM
                accumulation group per bank (hardware `start` clears the whole
                bank, so groups must not interleave within one), normalized
                immediately so the pool slot recycles."""
                h, it = tiles[idx]
                pair, hp = h // 2, h % 2
                if hp == 0 and qb == 0:
                    ost_tiles[(pair, it)] = ostp.tile(
                        [128, 4, 128], BF16, tag="ost", name=f"ost{pair}{it}")
                ost = ost_tiles[(pair, it)]
                ot = mscp.tile([128, 65], F32, tag="ms", name=f"ot{idx}_{qb}")
                for jt in range(NJT):
                    si, k = JT2SLAB[jt]
                    nc.tensor.matmul(
                        ot[:],
                        pt_slabs[(idx, si)][:, k, qb * 128:(qb + 1) * 128],
                        vaug[:, jt, h, :],
                        start=(jt == 0), stop=(jt == NJT - 1))
                rs = rsp.tile([128, 1], F32, tag="rs", name=f"rs{idx}{qb}")
                nc.vector.reciprocal(out=rs[:], in_=ot[:, 64:65])
                nc.vector.tensor_scalar_mul(
                    ost[:, qb, hp * 64:(hp + 1) * 64], ot[:, 0:64], rs[:])

            def chunk_transposes(idx, qbs=range(4)):
                h, it = tiles[idx]
                if h % 2 != 1:
                    return
                pair = h // 2
                ost = ost_tiles[(pair, it)]
                for qb in qbs:
                    tp = mscp.tile([128, 128], BF16, tag="ms", name=f"tp{idx}{qb}")
                    nc.tensor.transpose(tp[:], ost[:, qb, :], idn[:])
                    nc.vector.tensor_copy(
                        otn[:, pair, it * 512 + qb * 128:it * 512 + (qb + 1) * 128],
                        tp[:])

            def outproj_ct(it, ct, q0=0, qn=512):
                ps = mscp.tile([128, 512], F32, tag="ms", name=f"psp{it}{ct}")
                for r in range(2):
                    nc.tensor.matmul(
                        ps[:, 0:qn],
                        wp_sb[:, r, ct * 128:(ct + 1) * 128],
                        otn[:, r, it * 512 + q0:it * 512 + q0 + qn],
                        start=(r == 0), stop=(r == 1))
                o_sb = osbp.tile([128, 512], BF16, tag="osb", name=f"osb{it}{ct}")
                nc.vector.tensor_copy(o_sb[:, 0:qn], ps[:, 0:qn])
                nc.sync.dma_start(
                    out=outT[ct * 128:(ct + 1) * 128,
                             it * 512 + q0:it * 512 + q0 + qn],
                    in_=o_sb[:, 0:qn])

            # output projections are deferred (one 128-column piece at a
            # time) into the following tiles' chunk slots so the ACT-feeding
            # score matmuls of heavy tiles aren't crowded out
            defer = []

            def chunk_deferred():
                if defer:
                    defer.pop(0)()

            def epilogue_chunks(idx):
                h, it = tiles[idx]
                odd = h % 2 == 1
                pair = h // 2

                def s4():
                    chunk_transposes(idx, (0, 1))
                    if odd and it + 1 < NIT:
                        qproj_half(it + 1, pair, 0)
                    if not odd:
                        chunk_deferred()

                def s5():
                    chunk_transposes(idx, (2, 3))
                    if odd and it + 1 < NIT:
                        qproj_half(it + 1, pair, 1)
                    if h == 3:
                        for ct in range(8):
                            defer.append(
                                lambda it=it, ct=ct: outproj_ct(it, ct))
                    chunk_deferred()

                chunks = [
                    lambda: chunk_av_qb(idx, 0),
                    lambda: (chunk_av_qb(idx, 1), chunk_deferred()),
                    lambda: (chunk_av_qb(idx, 2), chunk_deferred()),
                    lambda: chunk_av_qb(idx, 3),
                    s4,
                    s5,
                ]
                while len(chunks) < NSLAB:
                    chunks.append(lambda: chunk_deferred())
                return chunks

            # ---- lead-in: Q proj of window 0, K per kv quarter as it
            # arrives, first two tiles' score slabs right behind (the ACT
            # engine is the critical resource — feed it ASAP); V projections
            # are deferred/spread since vaug is first read only at AV(0)
            qproj_half(0, 0, 0)
            filler(50)
            kproj_p(0, 0)
            filler(40)
            qproj_half(0, 0, 1)
            filler(20)
            kproj_p(1, 0)
            # kt quarters needed per slab si: last jt of the slab / 2
            slab_qhi = [(g0 + g - 1) * 128 // 256 for (g0, g) in SLABS]
            vq_done = 0
            kq_done = 2
            for si in range(NSLAB - 1):
                # the K-proj -> scores chain feeds ACT (the critical engine);
                # boost its scheduler priority over the V-projection backfill
                with tc.high_priority(offset=3000):
                    while kq_done <= slab_qhi[si]:
                        kproj_p(kq_done, 0)
                        kq_done += 1
                    sc_slab(0, si)
                    sc_slab(1, si)
                while vq_done < min(kq_done, 2 * si + 2, NYQ):
                    vproj(vq_done)
                    vq_done += 1
            while kq_done <= slab_qhi[NSLAB - 1]:
                kproj_p(kq_done, 0)
                kq_done += 1
            while vq_done < NYQ:
                vproj(vq_done)
                vq_done += 1
            sc_slab(0, NSLAB - 1)        # last kv chunks
            chunk_av_qb(0, 0)
            chunk_av_qb(0, 1)
            chunk_av_qb(0, 2)
            chunk_av_qb(0, 3)
            sc_slab(1, NSLAB - 1)
            # pair-1 projections (needed from tile 2 on): Q at lead end,
            # K woven just ahead of the tile-2 slabs that consume them
            qproj_half(0, 1, 0)
            qproj_half(0, 1, 1)
            nlead = int(_os.environ.get("K_KP1LEAD", "0"))
            for q in range(nlead):
                kproj_p(q, 1)
            if NSLAB == 6:
                full = {(2, 0): [0, 1], (2, 1): [2], (2, 2): [3, 4],
                        (2, 3): [5], (2, 4): [6, 7]}
            else:
                full = {(2, 0): [0, 1]}
                for _si in range(1, 7):
                    full[(2, _si)] = [_si + 1]
            kp1_sched = {}
            for _k, _qs in full.items():
                rem = [q for q in _qs if q >= nlead]
                if rem:
                    kp1_sched[_k] = rem

            # ---- steady-state: weave tile idx-1's epilogue chunks between
            # tile idx's score slabs
            for idx in range(2, len(tiles) + 1):
                chunks = epilogue_chunks(idx - 1)
                for si in range(NSLAB):
                    for q in kp1_sched.get((idx, si), []):
                        kproj_p(q, 1)
                    if idx < len(tiles):
                        sc_slab(idx, si)
                    chunks[si]()
            while defer:
                defer.pop(0)()
            if debug_taps:
                nc.sync.dma_start(out=taps["dbg_qt"], in_=qt[:])
                nc.sync.dma_start(out=taps["dbg_kt"], in_=kt[:])
                nc.sync.dma_start(out=taps["dbg_vaug"], in_=vaug[:])
                nc.sync.dma_start(out=taps["dbg_otn"], in_=otn[:])

    nc.compile()
    return nc


_NC_CACHE = {}


def _get_nc():
    if "nc" not in _NC_CACHE:
        _NC_CACHE["nc"] = build_kernel()
    return _NC_CACHE["nc"]


def _bf16(a):
    """Round-to-nearest-even f32 -> bf16 (stored as uint16 bit pattern
    viewed via numpy's void trick is messy; ml_dtypes handles it)."""
    import ml_dtypes
    return np.asarray(a, dtype=np.float32).astype(ml_dtypes.bfloat16)


def make_in_maps(x, y, Wq, Wkv, Wproj):
    """Host-side sharding: core = b * 4 + hg (hg = 4-head group)."""
    x = np.asarray(x, dtype=np.float32)
    y = np.asarray(y, dtype=np.float32)
    Wq = np.asarray(Wq, dtype=np.float32)
    Wkv = np.asarray(Wkv, dtype=np.float32).reshape(CTX, 2, H, DK)
    Wproj = np.asarray(Wproj, dtype=np.float32)
    ident = np.eye(128, dtype=np.float32)

    in_maps = []
    for core in range(8):
        b, hg = core // 4, core % 4
        hs = slice(4 * hg, 4 * hg + 4)
        in_maps.append({
            "xT": _bf16(np.ascontiguousarray(x[b].T)),
            "yT": _bf16(np.ascontiguousarray(y[b].T)),
            "wq": _bf16(np.ascontiguousarray(Wq[:, 4 * hg * DK:(4 * hg + 4) * DK])),
            "wk": _bf16(np.ascontiguousarray(Wkv[:, 0, hs, :].reshape(CTX, 4 * DK))),
            "wv": _bf16(np.ascontiguousarray(Wkv[:, 1, hs, :].reshape(CTX, 4 * DK))),
            "wp": _bf16(np.ascontiguousarray(Wproj[4 * hg * DK:(4 * hg + 4) * DK, :])),
            "ident": _bf16(ident),
        })
    return in_maps


def kernel(x, y, Wq, Wkv, Wproj, bproj):
    nc = _get_nc()
    in_maps = make_in_maps(x, y, Wq, Wkv, Wproj)
    res = run_bass_kernel_spmd(nc, in_maps, core_ids=list(range(8)))
    bproj = np.asarray(bproj, dtype=np.float32)
    out = np.empty((B, LQ, C), dtype=np.float32)
    for b in range(B):
        acc = res.results[4 * b]["outT"].astype(np.float32)
        for hg in range(1, 4):
            acc = acc + res.results[4 * b + hg]["outT"].astype(np.float32)
        out[b] = acc.T + bproj
    return out


# revision 20
# speedup vs baseline: 1.0916x; 1.0108x over previous
"""CrossAttentionBlock kernel for 8 Trainium2 NeuronCores.

Reference computation (per batch b):
    q = x @ Wq;  k,v = y @ Wkv;  per head: softmax(q k^T / sqrt(dk)) v;
    out = concat_heads @ Wproj + bproj

Sharding: 8 cores = 2 batches x 4 head-groups (4 heads each). Each core
computes the partial output contribution of its 4 heads for its batch;
the host sums the 4 partials per batch and adds the bias.

All device I/O is bf16 (host converts inputs, sums bf16 output partials
in f32): halves DMA traffic vs f32 and lets every matmul run at the
1-cycle/row bf16 rate.

Per-core pipeline (16 attention tiles = 4 heads x 4 query-windows of 512):
  scores S^T [keys128, q512] (bf16 matmuls) -> exp into P^T slabs: slab 0
  (kv chunks 0-1) via a one-instruction Schraudolph on the DVE
  (i16 = round(s*SCALE*128/ln2 + 16248.5), bitcast bf16 == exp approx),
  the rest on the ACT engine (the near-critical resource) -> AV in
  query-major bf16: out[q128, 65] = P^T-slice.T @ [V | ones] with row-sums
  landing in column 64 for free; one PSUM accumulation group per bank
  (hardware `start` clears the whole bank) normalized immediately via DVE
  reciprocal + per-partition tensor_scalar -> bf16 PE transpose (via a
  DMA'd identity) back to feature-major -> bf16 output projection whose
  PSUM->SBUF copies alternate between ACT and DVE, partials summed on the
  host. Software-pipelined: each tile's epilogue is chunked and woven
  between the next tile's score slabs so ACT never starves; a PE warm-up
  filler keeps the sim's p-state warm through the DMA-bound lead-in.
"""

import numpy as np

import concourse.bass as bass
import concourse.tile as tile
from concourse import bacc, mybir
from concourse.bass_utils import run_bass_kernel_spmd

B, LQ, LKV = 2, 2048, 2048
C, CTX, H, DK = 1024, 768, 16, 64
SCALE = DK ** (-0.5)
HD = 256                 # head-group width (4 heads x 64)

F32 = mybir.dt.float32
BF16 = mybir.dt.bfloat16
I16 = mybir.dt.int16

# Schraudolph exp: bitcast(round(s*SA + SB)) as bf16 ~= exp(s*SCALE)
SA = SCALE * 128.0 / float(np.log(2.0))
SB = 127.0 * 128.0 - 7.5

NCC = C // 128           # x contraction chunks (8)
NCTX = CTX // 128        # y contraction chunks (6)
NIT = LQ // 512          # query windows (4)
NYQ = LKV // 256         # kv quarter-windows (8)
NJT = LKV // 128         # kv chunks (16)
# exp slab schedule over the 16 kv chunks: (start, len)
import os as _os
if _os.environ.get("K_SLAB8"):
    SLABS = [(2 * i, 2) for i in range(8)]
    STP_BUFS = 3
    STP_CH = 2
else:
    SLABS = [(0, 2), (2, 3), (5, 3), (8, 3), (11, 3), (14, 2)]
    STP_BUFS = 2
    STP_CH = 3
NSLAB = len(SLABS)
# tiles 0/1 use finer slabs: the lead-in is ring-latency-bound (2-buf
# ping-pong), so smaller slabs shorten the scores->exp->scores period and
# get the ACT engine saturated sooner
SLABS01 = [(2 * i, 2) for i in range(8)]
DVE_SLABS = tuple(int(c) for c in _os.environ.get("K_DVE_SLABS", ""))
OSB_ACT = tuple(int(c) for c in _os.environ.get("K_OSB_ACT", ""))


def build_kernel(debug_taps=False):
    nc = bacc.Bacc("TRN2", target_bir_lowering=False, debug=False)

    xT = nc.dram_tensor("xT", [C, LQ], BF16, kind="ExternalInput").ap()
    yT = nc.dram_tensor("yT", [CTX, LKV], BF16, kind="ExternalInput").ap()
    wq = nc.dram_tensor("wq", [C, HD], BF16, kind="ExternalInput").ap()
    wk = nc.dram_tensor("wk", [CTX, HD], BF16, kind="ExternalInput").ap()
    wv = nc.dram_tensor("wv", [CTX, HD], BF16, kind="ExternalInput").ap()
    wp = nc.dram_tensor("wp", [HD, C], BF16, kind="ExternalInput").ap()
    idn_d = nc.dram_tensor("ident", [128, 128], BF16, kind="ExternalInput").ap()
    outT = nc.dram_tensor("outT", [C, LQ], BF16, kind="ExternalOutput").ap()
    taps = {}
    if debug_taps:
        taps["dbg_qt"] = nc.dram_tensor(
            "dbg_qt", [128, 2, LQ], BF16, kind="ExternalOutput").ap()
        taps["dbg_kt"] = nc.dram_tensor(
            "dbg_kt", [128, 2, LKV], BF16, kind="ExternalOutput").ap()
        taps["dbg_vaug"] = nc.dram_tensor(
            "dbg_vaug", [128, NJT, 4, 65], BF16, kind="ExternalOutput").ap()
        taps["dbg_otn"] = nc.dram_tensor(
            "dbg_otn", [128, 2, LQ], BF16, kind="ExternalOutput").ap()

    with tile.TileContext(nc) as tc:
        with (
            tc.tile_pool(name="wts", bufs=1) as wts,
            tc.tile_pool(name="acts", bufs=1) as acts,
            tc.tile_pool(name="xp", bufs=2) as xp,
            tc.tile_pool(name="yp", bufs=8) as yp,
            tc.tile_pool(name="ptp", bufs=18) as ptp,
            tc.tile_pool(name="ostp", bufs=2) as ostp,
            tc.tile_pool(name="rsp", bufs=4) as rsp,
            tc.tile_pool(name="osbp", bufs=4) as osbp,
            tc.tile_pool(name="stp", bufs=STP_BUFS, space="PSUM") as stp,
            tc.tile_pool(name="mscp", bufs=2, space="PSUM") as mscp,
        ):
            # ---- persistent weights / activations
            wq_sb = wts.tile([128, NCC, HD], BF16, tag="wq")
            wk_sb = wts.tile([128, NCTX, HD], BF16, tag="wk")
            wv_sb = wts.tile([128, NCTX, HD], BF16, tag="wv")
            wp_sb = wts.tile([128, 2, C], BF16, tag="wp")
            idn = wts.tile([128, 128], BF16, tag="idn")
            ones_sb = wts.tile([128, NJT, 4], BF16, tag="ones")
            dummy = wts.tile([128, 128], BF16, tag="dummy")

            qt = acts.tile([128, 2, LQ], BF16, tag="qt")       # Q^T pair-stacked
            kt = acts.tile([128, 2, LKV], BF16, tag="kt")      # K^T pair-stacked
            vaug = acts.tile([128, NJT, 4, 65], BF16, tag="vaug")  # [V_h | 1]
            otn = acts.tile([128, 2, LQ], BF16, tag="otn")     # normalized O^T

            # pin the Exp act-table + bias const load to t~0 (their DMAs
            # would otherwise queue behind all the input loads)
            nc.gpsimd.memset(dummy[:], 0.0)
            nc.scalar.activation(
                dummy[:, 0:1], dummy[:, 1:2],
                mybir.ActivationFunctionType.Exp, scale=SCALE)
            nc.gpsimd.memset(ones_sb[:], 1.0)
            nc.gpsimd.tensor_copy(
                vaug[:, :, :, 64:65],
                ones_sb[:].rearrange("p j (h o) -> p j h o", o=1))

            # ---- input DMAs in priority order (SP queue drains in order);
            # tiles 0/1 are pair-0 heads, so pair-0 weight halves come first
            wq_r = wq.rearrange("(cc p) h -> p cc h", p=128)
            wk_r = wk.rearrange("(cc p) h -> p cc h", p=128)
            nc.sync.dma_start(out=wq_sb, in_=wq_r)

            def load_x(w, half=None):
                t = xp.tile([128, NCC, 512], BF16, tag="x", name=f"x{w}")
                src = xT.rearrange("(cc p) l -> p cc l", p=128)
                if half is None:
                    nc.sync.dma_start(
                        out=t, in_=src[:, :, w * 512:(w + 1) * 512])
                else:
                    nc.sync.dma_start(
                        out=t[:, :, 0:256],
                        in_=src[:, :, w * 512:w * 512 + 256])
                return t

            def load_x_half2(t, w):
                src = xT.rearrange("(cc p) l -> p cc l", p=128)
                nc.sync.dma_start(
                    out=t[:, :, 256:512],
                    in_=src[:, :, w * 512 + 256:(w + 1) * 512])

            def load_yq(q):
                t = yp.tile([128, NCTX, 256], BF16, tag="y", name=f"y{q}")
                nc.sync.dma_start(
                    out=t,
                    in_=yT.rearrange("(cc p) l -> p cc l", p=128)
                    [:, :, q * 256:(q + 1) * 256])
                return t

            x_t = [None] * NIT
            y_q = [None] * NYQ
            x_t[0] = load_x(0, half=0)
            nc.sync.dma_start(out=wk_sb, in_=wk_r)
            y_q[0] = load_yq(0)
            load_x_half2(x_t[0], 0)
            y_q[1] = load_yq(1)
            nc.sync.dma_start(
                out=wv_sb, in_=wv.rearrange("(cc p) h -> p cc h", p=128))
            for q in range(2, NYQ):
                y_q[q] = load_yq(q)
            nc.sync.dma_start(out=idn, in_=idn_d)
            x_t[1] = load_x(1)
            nc.sync.dma_start(
                out=wp_sb, in_=wp.rearrange("(r p) o -> p r o", p=128))
            x_t[2] = load_x(2)
            x_t[3] = load_x(3)

            # ---- PE warm-up filler: keeps the PE busy stretch alive through
            # the DMA-bound lead-in so real bursts are costed at full p-state
            dps = stp.tile([128, STP_CH, 512], F32, tag="st", name="dps")

            def filler(n, gate=None):
                src = dummy[0:64, 0:64] if gate is None else gate
                for _ in range(n):
                    nc.tensor.matmul(
                        dps[0:64, 0, 0:64], src, src,
                        start=True, stop=True, skip_group_check=True)

            filler(60)

            # ---- projection helpers (kv projections run per quarter-window)
            def kproj_p(q, pair):
                ps = mscp.tile([128, 256], F32, tag="ms", name=f"psk{q}{pair}")
                for cc in range(NCTX):
                    nc.tensor.matmul(
                        ps[:],
                        wk_sb[:, cc, pair * 128:(pair + 1) * 128],
                        y_q[q][:, cc, :],
                        start=(cc == 0), stop=(cc == NCTX - 1))
                nc.vector.tensor_copy(kt[:, pair, q * 256:(q + 1) * 256], ps[:])

            def vproj(q):
                for j in range(2):
                    jt = 2 * q + j
                    ps = mscp.tile([128, 256], F32, tag="ms", name=f"psv{jt}")
                    for cc in range(NCTX):
                        nc.tensor.matmul(
                            ps[:],
                            y_q[q][:, cc, j * 128:(j + 1) * 128],
                            wv_sb[:, cc, :],
                            start=(cc == 0), stop=(cc == NCTX - 1))
                    nc.vector.tensor_copy(
                        vaug[:, jt, :, 0:64],
                        ps[:].rearrange("p (h d) -> p h d", d=64))

            def qproj_half(it, pair, half):
                ps = mscp.tile([128, 256], F32, tag="ms", name=f"psqh{pair}{half}")
                for cc in range(NCC):
                    nc.tensor.matmul(
                        ps[:],
                        wq_sb[:, cc, pair * 128:(pair + 1) * 128],
                        x_t[it][:, cc, half * 256:half * 256 + 256],
                        start=(cc == 0), stop=(cc == NCC - 1))
                nc.vector.tensor_copy(
                    qt[:, pair, it * 512 + half * 256:it * 512 + half * 256 + 256],
                    ps[:])

            def qproj_pair(it, pair):
                ps = mscp.tile([128, 512], F32, tag="ms", name=f"psq{it}{pair}")
                for cc in range(NCC):
                    nc.tensor.matmul(
                        ps[:],
                        wq_sb[:, cc, pair * 128:(pair + 1) * 128],
                        x_t[it][:, cc, :],
                        start=(cc == 0), stop=(cc == NCC - 1))
                nc.vector.tensor_copy(qt[:, pair, it * 512:(it + 1) * 512], ps[:])

            # ---- attention stages
            tiles = [(h, it) for it in range(NIT) for h in range(4)]
            ptj = {}          # (idx, jt) -> [128, 512] bf16 P^T row AP
            ot_tiles = {}
            ost_tiles = {}

            def tile_slabs(idx):
                return SLABS01 if idx < 2 else SLABS

            def sc_slab(idx, si):
                """Scores + exp for slab si of tile idx. DVE_SLABS' exp runs
                on the DVE via Schraudolph; the rest on ACT. Runs at boosted
                scheduler priority: these feed the near-saturated engines."""
                h, it = tiles[idx]
                pair, hp = h // 2, h % 2
                base = hp * 64
                g0, glen = tile_slabs(idx)[si]
                with tc.high_priority(offset=8000):
                    st = stp.tile([128, STP_CH, 512], F32, tag="st",
                                  name=f"st{idx}_{si}")
                    for k in range(glen):
                        jt = g0 + k
                        nc.tensor.matmul(
                            st[:, k, :],
                            kt[base:base + 64, pair, jt * 128:(jt + 1) * 128],
                            qt[base:base + 64, pair, it * 512:(it + 1) * 512],
                            start=True, stop=True)
                    if si in DVE_SLABS and idx >= 2:
                        pt = ptp.tile([128, STP_CH, 512], I16, tag="pt",
                                      name=f"pt{idx}_{si}")
                        nc.vector.tensor_scalar(
                            pt[:, 0:glen, :], st[:, 0:glen, :], SA, SB,
                            mybir.AluOpType.mult, mybir.AluOpType.add)
                        pta = pt[:].bitcast(BF16)
                    else:
                        pt = ptp.tile([128, STP_CH, 512], BF16, tag="pt",
                                      name=f"pt{idx}_{si}")
                        nc.scalar.activation(
                            pt[:, 0:glen, :], st[:, 0:glen, :],
                            mybir.ActivationFunctionType.Exp, scale=SCALE)
                        pta = pt[:]
                    for k in range(glen):
                        ptj[(idx, g0 + k)] = pta[:, k, :]

            def chunk_av_qb(idx, qb):
                """AV accumulation for one 128-query block: a single PSUM
                accumulation group per bank (hardware `start` clears the whole
                bank, so groups must not interleave within one), normalized
                immediately so the pool slot recycles."""
                h, it = tiles[idx]
                pair, hp = h // 2, h % 2
                if hp == 0 and qb == 0:
                    ost_tiles[(pair, it)] = ostp.tile(
                        [128, 4, 128], BF16, tag="ost", name=f"ost{pair}{it}")
                ost = ost_tiles[(pair, it)]
                ot = mscp.tile([128, 65], F32, tag="ms", name=f"ot{idx}_{qb}")
                for jt in range(NJT):
                    nc.tensor.matmul(
                        ot[:],
                        ptj[(idx, jt)][:, qb * 128:(qb + 1) * 128],
                        vaug[:, jt, h, :],
                        start=(jt == 0), stop=(jt == NJT - 1))
                rs = rsp.tile([128, 1], F32, tag="rs", name=f"rs{idx}{qb}")
                nc.vector.reciprocal(out=rs[:], in_=ot[:, 64:65])
                nc.vector.tensor_scalar_mul(
                    ost[:, qb, hp * 64:(hp + 1) * 64], ot[:, 0:64], rs[:])

            def chunk_transposes(idx, qbs=range(4)):
                h, it = tiles[idx]
                if h % 2 != 1:
                    return
                pair = h // 2
                ost = ost_tiles[(pair, it)]
                for qb in qbs:
                    tp = mscp.tile([128, 128], BF16, tag="ms", name=f"tp{idx}{qb}")
                    nc.tensor.transpose(tp[:], ost[:, qb, :], idn[:])
                    nc.vector.tensor_copy(
                        otn[:, pair, it * 512 + qb * 128:it * 512 + (qb + 1) * 128],
                        tp[:])

            def outproj_ct(it, ct, q0=0, qn=512):
                ps = mscp.tile([128, 512], F32, tag="ms", name=f"psp{it}{ct}")
                for r in range(2):
                    nc.tensor.matmul(
                        ps[:, 0:qn],
                        wp_sb[:, r, ct * 128:(ct + 1) * 128],
                        otn[:, r, it * 512 + q0:it * 512 + q0 + qn],
                        start=(r == 0), stop=(r == 1))
                o_sb = osbp.tile([128, 512], BF16, tag="osb", name=f"osb{it}{ct}")
                nc.vector.tensor_copy(o_sb[:, 0:qn], ps[:, 0:qn])
                nc.sync.dma_start(
                    out=outT[ct * 128:(ct + 1) * 128,
                             it * 512 + q0:it * 512 + q0 + qn],
                    in_=o_sb[:, 0:qn])

            # output projections are deferred (one 128-column piece at a
            # time) into the following tiles' chunk slots so the ACT-feeding
            # score matmuls of heavy tiles aren't crowded out
            defer = []

            def chunk_deferred():
                if defer:
                    defer.pop(0)()

            def epilogue_chunks(idx):
                h, it = tiles[idx]
                odd = h % 2 == 1
                pair = h // 2

                if idx == len(tiles) - 1:
                    # tail: interleave the last window's output projection
                    # with its AV/transposes at query-half granularity so the
                    # post-last-exp critical chain is as short as possible
                    def t2():
                        chunk_av_qb(idx, 2)
                        chunk_transposes(idx, (0, 1))
                    def t3():
                        chunk_av_qb(idx, 3)
                        for ct in range(8):
                            outproj_ct(it, ct, 0, 256)
                    def t4():
                        chunk_transposes(idx, (2, 3))
                        for ct in range(4):
                            outproj_ct(it, ct, 256, 256)
                    def t5():
                        for ct in range(4, 8):
                            outproj_ct(it, ct, 256, 256)
                    chunks = [
                        lambda: chunk_av_qb(idx, 0),
                        lambda: chunk_av_qb(idx, 1),
                        t2, t3, t4, t5,
                    ]
                    while len(chunks) < NSLAB:
                        chunks.append(lambda: None)
                    return chunks

                def s4():
                    chunk_transposes(idx, (0, 1))
                    if odd and it + 1 < NIT:
                        qproj_half(it + 1, pair, 0)
                    if not odd:
                        chunk_deferred()

                def s5():
                    chunk_transposes(idx, (2, 3))
                    if odd and it + 1 < NIT:
                        qproj_half(it + 1, pair, 1)
                    if h == 3:
                        for ct in range(8):
                            defer.append(
                                lambda it=it, ct=ct: outproj_ct(it, ct))
                    chunk_deferred()

                chunks = [
                    lambda: chunk_av_qb(idx, 0),
                    lambda: (chunk_av_qb(idx, 1), chunk_deferred()),
                    lambda: (chunk_av_qb(idx, 2), chunk_deferred()),
                    lambda: chunk_av_qb(idx, 3),
                    s4,
                    s5,
                ]
                while len(chunks) < NSLAB:
                    chunks.append(lambda: chunk_deferred())
                return chunks

            # ---- lead-in: Q proj of window 0, K per kv quarter just ahead
            # of the (finer) tile-0/1 slabs that consume it — the ACT engine
            # is the critical resource, feed it ASAP; V projections are
            # spread since vaug is first read only at AV(0)
            NS0 = len(SLABS01)
            qproj_half(0, 0, 0)
            filler(50)
            kproj_p(0, 0)
            filler(40)
            qproj_half(0, 0, 1)
            filler(20)
            kproj_p(1, 0)
            # kt quarters needed per slab si: last jt of the slab / 2
            slab_qhi0 = [(g0 + g - 1) * 128 // 256 for (g0, g) in SLABS01]
            vq_done = 0
            kq_done = 2
            for si in range(NS0 - 1):
                # the K-proj -> scores chain feeds ACT (the critical engine);
                # boost its scheduler priority over the V-projection backfill
                with tc.high_priority(offset=3000):
                    while kq_done <= slab_qhi0[si]:
                        kproj_p(kq_done, 0)
                        kq_done += 1
                    sc_slab(0, si)
                    sc_slab(1, si)
                while vq_done < min(kq_done, si + 1, NYQ):
                    vproj(vq_done)
                    vq_done += 1
            while kq_done <= slab_qhi0[NS0 - 1]:
                kproj_p(kq_done, 0)
                kq_done += 1
            while vq_done < NYQ:
                vproj(vq_done)
                vq_done += 1
            sc_slab(0, NS0 - 1)        # last kv chunks
            chunk_av_qb(0, 0)
            chunk_av_qb(0, 1)
            chunk_av_qb(0, 2)
            chunk_av_qb(0, 3)
            sc_slab(1, NS0 - 1)
            # pair-1 projections (needed from tile 2 on): Q at lead end,
            # K woven just ahead of the tile-2 slabs that consume them
            qproj_half(0, 1, 0)
            qproj_half(0, 1, 1)
            nlead = int(_os.environ.get("K_KP1LEAD", "0"))
            for q in range(nlead):
                kproj_p(q, 1)
            if NSLAB == 6:
                full = {(2, 0): [0, 1], (2, 1): [2], (2, 2): [3, 4],
                        (2, 3): [5], (2, 4): [6, 7]}
            else:
                full = {(2, 0): [0, 1]}
                for _si in range(1, 7):
                    full[(2, _si)] = [_si + 1]
            kp1_sched = {}
            for _k, _qs in full.items():
                rem = [q for q in _qs if q >= nlead]
                if rem:
                    kp1_sched[_k] = rem

            # ---- steady-state: weave tile idx-1's epilogue chunks between
            # tile idx's score slabs
            for idx in range(2, len(tiles) + 1):
                chunks = epilogue_chunks(idx - 1)
                for si in range(NSLAB):
                    for q in kp1_sched.get((idx, si), []):
                        kproj_p(q, 1)
                    if idx < len(tiles):
                        sc_slab(idx, si)
                    chunks[si]()
            while defer:
                defer.pop(0)()
            if debug_taps:
                nc.sync.dma_start(out=taps["dbg_qt"], in_=qt[:])
                nc.sync.dma_start(out=taps["dbg_kt"], in_=kt[:])
                nc.sync.dma_start(out=taps["dbg_vaug"], in_=vaug[:])
                nc.sync.dma_start(out=taps["dbg_otn"], in_=otn[:])

    nc.compile()
    return nc


_NC_CACHE = {}


def _get_nc():
    if "nc" not in _NC_CACHE:
        _NC_CACHE["nc"] = build_kernel()
    return _NC_CACHE["nc"]


def _bf16(a):
    """Round-to-nearest-even f32 -> bf16 (stored as uint16 bit pattern
    viewed via numpy's void trick is messy; ml_dtypes handles it)."""
    import ml_dtypes
    return np.asarray(a, dtype=np.float32).astype(ml_dtypes.bfloat16)


def make_in_maps(x, y, Wq, Wkv, Wproj):
    """Host-side sharding: core = b * 4 + hg (hg = 4-head group)."""
    x = np.asarray(x, dtype=np.float32)
    y = np.asarray(y, dtype=np.float32)
    Wq = np.asarray(Wq, dtype=np.float32)
    Wkv = np.asarray(Wkv, dtype=np.float32).reshape(CTX, 2, H, DK)
    Wproj = np.asarray(Wproj, dtype=np.float32)
    ident = np.eye(128, dtype=np.float32)

    in_maps = []
    for core in range(8):
        b, hg = core // 4, core % 4
        hs = slice(4 * hg, 4 * hg + 4)
        in_maps.append({
            "xT": _bf16(np.ascontiguousarray(x[b].T)),
            "yT": _bf16(np.ascontiguousarray(y[b].T)),
            "wq": _bf16(np.ascontiguousarray(Wq[:, 4 * hg * DK:(4 * hg + 4) * DK])),
            "wk": _bf16(np.ascontiguousarray(Wkv[:, 0, hs, :].reshape(CTX, 4 * DK))),
            "wv": _bf16(np.ascontiguousarray(Wkv[:, 1, hs, :].reshape(CTX, 4 * DK))),
            "wp": _bf16(np.ascontiguousarray(Wproj[4 * hg * DK:(4 * hg + 4) * DK, :])),
            "ident": _bf16(ident),
        })
    return in_maps


def kernel(x, y, Wq, Wkv, Wproj, bproj):
    nc = _get_nc()
    in_maps = make_in_maps(x, y, Wq, Wkv, Wproj)
    res = run_bass_kernel_spmd(nc, in_maps, core_ids=list(range(8)))
    bproj = np.asarray(bproj, dtype=np.float32)
    out = np.empty((B, LQ, C), dtype=np.float32)
    for b in range(B):
        acc = res.results[4 * b]["outT"].astype(np.float32)
        for hg in range(1, 4):
            acc = acc + res.results[4 * b + hg]["outT"].astype(np.float32)
        out[b] = acc.T + bproj
    return out
